# revision 38
# baseline (speedup 1.0000x reference)
"""Trainium2 Bass kernel for nn_Attention_79121887527485.

Multi-head causal attention with ALiBi, B=2 S=2048 D=2048 H=16 DH=128.
Tensor-parallel over heads across 8 NeuronCores: core c owns heads
c, c+8 (rows of Wq/Wk/Wv, cols of Wo). Each core computes a full
[BS, D] partial of the output projection; the host sums the 8 partials
(the unshard step for the input-sharded Wo).

v2: fine-grained emission weaving. The three per-chunk phases (QKV
projection, attention, output projection) are built as step lists and
woven proportionally so the PE stream always has independent matmuls
to hide the score->mask->exp->PV dependency latency. Softmax
denominators accumulate on DVE in fp16 (ptsum) with one ones-matmul
per chunk, freeing PSUM banks for deeper score pipelining. Slot1
causal mask adds touch only the first 128 columns (the causal pattern
is zero beyond). Scalar engine runs exp exclusively; copies live on
gpsimd/vector.
"""

import math
from contextlib import ExitStack

import numpy as np
import ml_dtypes

import concourse.bass as bass
import concourse.bacc as bacc
import concourse.tile as tile
from concourse import mybir
from concourse.bass_utils import run_bass_kernel_spmd

B, S, D, H, DH = 2, 2048, 2048, 16, 128
NSC_G = 8  # global 512-col s-chunks over batch*seq
NCORES = 8
HL = H // NCORES          # 2 local heads per core
BS = B * S                # 4096
HD = HL * DH              # 256 local head dims per core
SCALE = 1.0 / math.sqrt(DH)
C0 = 14.0                 # bound for scale*raw_score (empirical max ~8.7)
NEG = -1.0e6              # raw-units additive causal mask (-8.8e4 after scale)

F32 = mybir.dt.float32
BF16 = mybir.dt.bfloat16
F16 = mybir.dt.float16

_SLOPES = [2.0 ** (-(i + 1) / 2.0) for i in range(H)]

# core c owns heads (c, c + 8): local slot lh=0 covers heads 0-7, lh=1
# covers heads 8-15. ALiBi decay lets the program skip k-tiles whose
# whole contribution is < e^-DROP_T relative; the skip set must be the
# union over cores, so it is governed by the smallest slope in each slot.
DROP_T = 5.0
_SLOT_MIN_SLOPE = [_SLOPES[7], _SLOPES[15]]

ND = D // 128   # 16 d-tiles
NQC = S // 512  # 4 q-chunks per batch


def _heads(c):
    return [c, c + 8]


def _kept_kts(lh, qc):
    kts = []
    for kt in range(4 * qc + 4):
        dist = qc * 512 - (kt * 128 + 127)
        if dist > 0 and _SLOT_MIN_SLOPE[lh] * dist > DROP_T:
            continue
        kts.append(kt)
    return kts


def _weave(streams):
    """streams: list of lists of (weight, thunk). Emit all thunks, each
    stream in order, interleaved by cumulative-weight fraction."""
    tagged = []
    for si, steps in enumerate(streams):
        tot = float(sum(w for w, _ in steps)) or 1.0
        cum = 0.0
        for i, (w, fn) in enumerate(steps):
            cum += w
            tagged.append((cum / tot, si, i, fn))
    tagged.sort(key=lambda t: (t[0], t[1], t[2]))
    for _, _, _, fn in tagged:
        fn()


def _build_nc() -> bass.Bass:
    nc = bacc.Bacc("TRN2", target_bir_lowering=False, debug=False, num_devices=NCORES)

    xt_d = nc.dram_tensor("xt", [NSC_G, 128, 8192], BF16, kind="ExternalInput")
    wq_d = nc.dram_tensor("wq_t", [128, (D // 128) * HD], BF16, kind="ExternalInput")
    wk_d = nc.dram_tensor("wk_t", [128, (D // 128) * HD], BF16, kind="ExternalInput")
    wv_d = nc.dram_tensor("wv_t", [128, (D // 128) * HD], BF16, kind="ExternalInput")
    wo_d = nc.dram_tensor("wo_t", [128, HL * D], BF16, kind="ExternalInput")
    # mask: [:, :512] slot0 slope-mask (+causal), [:, 512:] causal-only
    mask_d = nc.dram_tensor("mask", [128, 2 * 512], F32, kind="ExternalInput")
    qrow_d = nc.dram_tensor("qrow", [128, 4 * 512], F32, kind="ExternalInput")
    # kbias: 0:16 slot0 j<0 per-kt, 16 slot0 diag, 17+qc*16+kt slot1
    kbias_d = nc.dram_tensor("kbias", [128, 17 + 64], F32, kind="ExternalInput")
    # out layout [sc, p, ot, c]: out[ot*128+p, sc*512+c] = out_t[sc,p,ot,c]
    out_d = nc.dram_tensor("out_t", [NSC_G, 128, 16, 512], F16,
                           kind="ExternalOutput")

    with tile.TileContext(nc) as tc, ExitStack() as ctx:
        const = ctx.enter_context(tc.tile_pool(name="const", bufs=1))
        xt_pool = ctx.enter_context(tc.tile_pool(name="xt", bufs=2))
        pt_pool = ctx.enter_context(tc.tile_pool(name="pt", bufs=10))
        pts_pool = ctx.enter_context(tc.tile_pool(name="pts", bufs=4))
        rc_pool = ctx.enter_context(tc.tile_pool(name="rc", bufs=2))
        oe_pool = ctx.enter_context(tc.tile_pool(name="oe", bufs=6))

        # ---- resident constants / weights ----
        wq_sb = const.tile([128, 16 * HD], BF16, tag="wq", name="wq")
        wk_sb = const.tile([128, 16 * HD], BF16, tag="wk", name="wk")
        wv_sb = const.tile([128, 16 * HD], BF16, tag="wv", name="wv")
        wo_sb = const.tile([128, HL * D], BF16, tag="wo")
        mask_sb = const.tile([128, 2 * 512], F32, tag="mask")
        qrow_sb = const.tile([128, 4 * 512], F32, tag="qrow")
        kbias_sb = const.tile([128, 17 + 64], F32, tag="kbias")
        ones_sb = const.tile([128, 128], BF16, tag="ones")

        # startup staging: only the first-needed transfers go upfront
        # (wq/wk quarter 0, xt chunk-0 pieces 0-1, the small attention
        # consts); everything else is woven into bracket 0's step stream
        # so concurrent-transfer contention stays low.
        def dma_w(which, h=None):
            # wk rides the gpsimd queue so the two 1MB weight streams
            # (wq on scalar, wk on gpsimd) transfer in parallel at start
            w_sb, w_d, eng = {"q": (wq_sb, wq_d, nc.scalar),
                              "k": (wk_sb, wk_d, nc.gpsimd),
                              "v": (wv_sb, wv_d, nc.scalar)}[which]
            if h is None:
                eng.dma_start(out=w_sb[:], in_=w_d.ap())
            else:
                eng.dma_start(
                    out=w_sb[:, h * 8 * HD:(h + 1) * 8 * HD],
                    in_=w_d.ap()[:, h * 8 * HD:(h + 1) * 8 * HD])

        def dma_wo():
            nc.gpsimd.dma_start(out=wo_sb[:], in_=wo_d.ap())

        nc.vector.memset(ones_sb[:], 1.0)

        # ---- fine-grained resident activations ----
        qt_sb = [[[const.tile([128, 512], BF16, tag=f"qt{lh}{b}{qc}", name=f"qt{lh}{b}{qc}")
                   for qc in range(NQC)] for b in range(B)] for lh in range(HL)]
        kt_sb = [[[const.tile([128, 512], BF16, tag=f"kt{lh}{b}{qc}", name=f"kt{lh}{b}{qc}")
                   for qc in range(NQC)] for b in range(B)] for lh in range(HL)]
        v_sb = [[const.tile([128, HD], BF16, tag=f"v{b}_{st}", name=f"v{b}_{st}")
                 for st in range(16)] for b in range(B)]
        zt_sb = [[[const.tile([128, 512], BF16, tag=f"zt{lh}{b}{qc}", name=f"zt{lh}{b}{qc}")
                   for qc in range(NQC)] for b in range(B)] for lh in range(HL)]

        with ExitStack() as pctx:
            ps_qkv = pctx.enter_context(tc.tile_pool(name="ps_qkv", bufs=2, space="PSUM"))
            ps_att = pctx.enter_context(tc.tile_pool(name="ps_att", bufs=3, space="PSUM"))
            ps_l = pctx.enter_context(tc.tile_pool(name="ps_l", bufs=1, space="PSUM"))
            ps_z = pctx.enter_context(tc.tile_pool(name="ps_z", bufs=2, space="PSUM"))

            cast_eng = [0]
            oe_eng = [0]
            n_out = [0]

            def emit_xt_dma(sc, piecewise, defer=False, eng=None):
                eng = eng or nc.sync
                halves = [
                    xt_pool.tile([128, 8 * 512], BF16, tag=f"xt{h}",
                                 name=f"xt_{sc}_{h}")
                    for h in range(2)
                ]
                if piecewise:
                    # 4 quarter transfers into the 2 half tiles: finer
                    # startup granularity, same addressing as steady mode
                    thunks = [
                        (lambda pp=p: eng.dma_start(
                            out=halves[pp // 2][:, (pp % 2) * 2048:
                                                (pp % 2) * 2048 + 2048],
                            in_=xt_d.ap()[sc, :, pp * 2048:(pp + 1) * 2048],
                        )) for p in range(4)
                    ]
                else:
                    thunks = [
                        (lambda hh=h: eng.dma_start(
                            out=halves[hh][:],
                            in_=xt_d.ap()[sc, :, hh * 4096:(hh + 1) * 4096],
                        )) for h in range(2)
                    ]

                def xt_sl(dt, lo, size):
                    half = halves[dt // 8]
                    base = (dt % 8) * 512 + lo
                    return half[:, base:base + size]
                if not defer:
                    for t in thunks:
                        t()
                return xt_sl, thunks

            def qkv_steps(b, scb, xt_sl, piecewise):
                """32 steps of ~4 matmuls each. piecewise: Q/K chains open
                together, progressing per 4-dt piece (chunk-0 DMA overlap)."""
                steps = []
                qk_state = {}

                def qk_piece(w_sb, dest, lh, pool, k, nk=2):
                    key = (id(dest), lh)
                    if k == 0:
                        qk_state[key] = pool.tile([128, 512], F32, tag="mm", name="qkpsum")
                    psum = qk_state[key]
                    for dt in range(nk * k, nk * k + nk):
                        w_sl = w_sb[:, dt * HD + lh * 128:
                                    dt * HD + lh * 128 + 128]
                        nc.tensor.matmul(
                            psum[:], w_sl, xt_sl(dt, 0, 512),
                            start=(dt == 0), stop=(dt == ND - 1),
                        )
                    if nk * k + nk == ND:
                        nc.vector.tensor_copy(dest[lh][b][scb][:], psum[:])

                def v_piece(ss, k, psum_holder):
                    if k == 0:
                        psum_holder[0] = ps_qkv.tile([128, HD], F32, tag="mm", name="vpsum")
                    psum = psum_holder[0]
                    for dt in range(2 * k, 2 * k + 2):
                        nc.tensor.matmul(
                            psum[:],
                            xt_sl(dt, ss * 128, 128),
                            wv_sb[:, dt * HD:dt * HD + HD],
                            start=(dt == 0), stop=(dt == ND - 1),
                        )
                    if k == 7:
                        nc.vector.tensor_copy(v_sb[b][scb * 4 + ss][:], psum[:])

                if piecewise:
                    # all four Q/K chains advance piece-by-piece as xt lands
                    for k in range(4):
                        for w_sb, dest, pool in ((wq_sb, qt_sb, ps_qkv),
                                                 (wk_sb, kt_sb, ps_att)):
                            for lh in range(HL):
                                steps.append((2048, lambda w=w_sb, d=dest, l=lh,
                                              p=pool, kk=k: qk_piece(w, d, l, p, kk, nk=4)))
                else:
                    for w_sb, dest in ((wq_sb, qt_sb), (wk_sb, kt_sb)):
                        for lh in range(HL):
                            for k in range(8):
                                steps.append((1024, lambda w=w_sb, d=dest, l=lh,
                                              kk=k: qk_piece(w, d, l, ps_qkv, kk)))
                for ss in range(4):
                    holder = [None]
                    for k in range(8):
                        steps.append((512, lambda s=ss, kk=k, h=holder:
                                      v_piece(s, kk, h)))
                return steps

            def attn_steps(b, qc):
                """One step per (lh, k-tile) plus a finalize per lh.
                Most chunks: l accumulates on PE via per-tile
                ones-matmuls (lpsum in ps_l), lh streams sequential to
                bound PSUM pressure. attn(0,2)/(0,3) (woven into
                qkv-rich brackets where DVE has slack but PE doesn't):
                l via fp32 ptsum adds on DVE + one ones-matmul; lh
                streams woven."""
                l_on_pe = not (b == 0 and qc >= 2)
                st = {}
                for lh in range(HL):
                    st[lh] = {
                        "kts": _kept_kts(lh, qc),
                        "zpsum": None, "ptsum": None, "lpsum": None,
                        "pending": [],
                    }

                def lz(lh, i, kt, lo, n, pt):
                    s = st[lh]
                    last = len(s["kts"]) - 1
                    if l_on_pe:
                        nc.tensor.matmul(
                            s["lpsum"][:, lo:512], ones_sb[:], pt[:, 0:n],
                            start=(i == 0), stop=(i == last),
                            skip_group_check=True,
                        )
                    nc.tensor.matmul(
                        s["zpsum"][:, lo:512],
                        v_sb[b][kt][:, lh * 128:(lh + 1) * 128],
                        pt[:, 0:n],
                        start=(i == 0), stop=(i == last),
                        skip_group_check=True,
                    )

                def tile_step(lh, i):
                    s = st[lh]
                    kt = s["kts"][i]
                    if i == 0:
                        s["zpsum"] = ps_z.tile([128, 512], F32, tag="z",
                                               name="zpsum")
                        if l_on_pe:
                            s["lpsum"] = ps_l.tile([128, 512], F32,
                                                   tag="lp", name="lpsum")
                        else:
                            s["ptsum"] = pts_pool.tile([128, 512], F32,
                                                       tag="pts", name="ptsum")
                    j = kt - 4 * qc
                    lo = 128 * j if j >= 0 else 0
                    n = 512 - lo
                    spsum = ps_att.tile([128, 512], F32, tag="mm", name="spsum")
                    nc.tensor.matmul(
                        spsum[:, 0:n],
                        kt_sb[lh][b][kt // 4][:, (kt % 4) * 128:(kt % 4) * 128 + 128],
                        qt_sb[lh][b][qc][:, lo:512],
                        start=True, stop=True,
                    )
                    causal_post = False
                    if lh == 0:
                        if j >= 0:
                            add_sl = mask_sb[:, 0:n]
                            bias_sl = kbias_sb[:, 16:17]
                        else:
                            add_sl = qrow_sb[:, qc * 512:(qc + 1) * 512]
                            bias_sl = kbias_sb[:, kt:kt + 1]
                        nc.vector.tensor_add(spsum[:, 0:n], spsum[:, 0:n], add_sl)
                    else:
                        causal_post = j >= 0
                        bc = 17 + qc * 16 + kt
                        bias_sl = kbias_sb[:, bc:bc + 1]
                    pt = pt_pool.tile([128, 512], BF16, tag="pt", name="pt")
                    nc.scalar.activation(
                        pt[:, 0:n], spsum[:, 0:n],
                        mybir.ActivationFunctionType.Exp,
                        bias=bias_sl,
                        scale=SCALE,
                    )
                    if causal_post:
                        # causal zero-fill of the p>c triangle; the causal
                        # pattern only lives in the first 128 columns
                        nc.gpsimd.affine_select(
                            pt[:, 0:128], pt[:, 0:128],
                            pattern=[[1, 128]],
                            compare_op=mybir.AluOpType.is_ge,
                            fill=0.0, base=0, channel_multiplier=-1,
                        )
                    if not l_on_pe:
                        if i == 0:
                            nc.vector.tensor_copy(s["ptsum"][:], pt[:])
                        else:
                            nc.vector.tensor_add(
                                s["ptsum"][:, lo:512], s["ptsum"][:, lo:512],
                                pt[:, 0:n])
                    s["pending"].append((lh, i, kt, lo, n, pt))
                    if len(s["pending"]) > 4:
                        lz(*s["pending"].pop(0))

                def finalize(lh):
                    s = st[lh]
                    for p in s["pending"]:
                        lz(*p)
                    s["pending"] = []
                    if l_on_pe:
                        lpsum = s["lpsum"]
                    else:
                        ptsb = rc_pool.tile([128, 512], BF16, tag="ptsb",
                                            name="ptsb")
                        nc.vector.tensor_copy(ptsb[:], s["ptsum"][:])
                        lpsum = ps_att.tile([128, 512], F32, tag="mm",
                                            name="lpsum")
                        nc.tensor.matmul(lpsum[:], ones_sb[:], ptsb[:],
                                         start=True, stop=True)
                    recip = rc_pool.tile([128, 512], F32, tag="rc", name="rc")
                    nc.vector.reciprocal_approx_fast(recip[:], lpsum[:])
                    nc.vector.tensor_mul(zt_sb[lh][b][qc][:], s["zpsum"][:],
                                         recip[:])

                streams = []
                for lh in range(HL):
                    kts = st[lh]["kts"]
                    sub = []
                    for i in range(len(kts)):
                        j = kts[i] - 4 * qc
                        n = 512 - (128 * j if j >= 0 else 0)
                        wgt = (3 if l_on_pe else 2) * n + 150
                        sub.append((wgt, lambda l=lh, ii=i: tile_step(l, ii)))
                    sub.append((300, lambda l=lh: finalize(l)))
                    streams.append(sub)
                if l_on_pe:
                    # lh sequential: bounds open-PSUM count (lpsum+spsum)
                    return streams[0] + streams[1]
                tagged = []
                for si, sub in enumerate(streams):
                    tot = float(sum(w for w, _ in sub))
                    cum = 0.0
                    for i, (w, fn) in enumerate(sub):
                        cum += w
                        tagged.append((cum / tot, si, i, (w, fn)))
                tagged.sort(key=lambda t: (t[0], t[1], t[2]))
                return [wf for _, _, _, wf in tagged]

            def outproj_steps(b, scb, oe_scalar=False, oe_vector=False,
                              tail=False):
                sc = b * NQC + scb
                steps = []
                state = {}

                npk = 2 if tail else 4

                def ot_step(pack, j):
                    ot = npk * pack + j
                    if j == 0:
                        state[pack] = oe_pool.tile([128, npk, 512], F16,
                                                   tag="oe", name="oe")
                    oe = state[pack]
                    psum = ps_att.tile([128, 512], F32, tag="mm", name="opsum")
                    for lh in range(HL):
                        nc.tensor.matmul(
                            psum[:],
                            wo_sb[:, lh * D + ot * 128: lh * D + ot * 128 + 128],
                            zt_sb[lh][b][scb][:],
                            start=(lh == 0), stop=(lh == HL - 1),
                        )
                    # during ptsum-heavy attention brackets DVE is loaded
                    # (oe_scalar); in exp-heavy tail brackets scalar is
                    # (oe_vector)
                    if oe_vector:
                        on_vec = oe_eng[0] % 4 != 0
                    elif oe_scalar:
                        on_vec = False
                    else:
                        on_vec = oe_eng[0] % 2 == 1
                    if on_vec:
                        nc.vector.tensor_copy(oe[:, j, :], psum[:])
                    else:
                        nc.scalar.copy(oe[:, j, :], psum[:])
                    oe_eng[0] += 1
                    if j == npk - 1:
                        if tail:
                            dma_eng = (nc.sync, nc.gpsimd, nc.scalar,
                                       nc.sync)[n_out[0] % 4]
                        else:
                            dma_eng = (nc.sync, nc.gpsimd)[n_out[0] % 2]
                        dma_eng.dma_start(
                            out=out_d.ap()[sc, :, npk * pack:npk * pack + npk, :],
                            in_=oe[:],
                        )
                        n_out[0] += 1

                for pack in range(16 // npk):
                    for j in range(npk):
                        steps.append((1024, lambda p=pack, jj=j: ot_step(p, jj)))
                return steps

            # ---- woven emission ----
            # bracket 0 is hand-ordered: the remaining startup DMAs are
            # staggered between chunk-0 compute steps so few transfers
            # contend at any time. Later brackets weave proportionally;
            # xt DMA for chunk sc+2 is emitted while chunk sc computes
            # (the trigger self-paces on the xt buffer WAR semaphore).
            # PE clock warmup: the p-state ramps with busy time, and the
            # first ~8us of real matmuls otherwise run at the low clock
            # while the startup DMAs stream in. Burn ~5us of dummy
            # matmuls (result never read) so the array is near full
            # clock when wq/xt land.
            warm = ps_z.tile([128, 128], F32, tag="z", name="warm")
            for wi in range(48):
                nc.tensor.matmul(warm[:], ones_sb[:], ones_sb[:],
                                 start=(wi == 0), stop=(wi == 47))

            xt_sl0, thunks0 = emit_xt_dma(0, piecewise=True, defer=True)
            xt_sl1, thunks1 = emit_xt_dma(1, piecewise=False, defer=True)
            xt_sl2, thunks2 = emit_xt_dma(2, piecewise=False, defer=True)
            dma_w("q", 0)
            dma_w("k", 0)
            thunks0[0]()
            steps0 = qkv_steps(0, 0, xt_sl0, piecewise=True)
            inserts = {
                1: [thunks0[1], lambda: dma_w("q", 1)],
                2: [lambda: dma_w("k", 1)],
                3: [thunks0[2]],
                6: [thunks0[3]],
                8: [lambda: dma_w("v")],
                12: [thunks1[0]],
                14: [lambda: nc.gpsimd.dma_start(out=mask_sb[:], in_=mask_d.ap()),
                     lambda: nc.gpsimd.dma_start(out=kbias_sb[:], in_=kbias_d.ap()),
                     lambda: nc.gpsimd.dma_start(out=qrow_sb[:], in_=qrow_d.ap())],
                16: [thunks1[1]],
                20: [dma_wo],
                24: [thunks2[0]],
                28: [thunks2[1]],
            }
            for idx, (w, fn) in enumerate(steps0):
                for dfn in inserts.get(idx, ()):
                    dfn()
                fn()

            xt_sls = {1: xt_sl1, 2: xt_sl2}
            brackets = [
                [("qkv", 0, 1, False)],
                [("qkv", 0, 2, False), ("attn", 0, 0)],
                [("qkv", 0, 3, False), ("attn", 0, 1), ("out", 0, 0, False)],
                [("qkv", 1, 0, False), ("attn", 0, 2), ("out", 0, 1, True)],
                [("qkv", 1, 1, False), ("attn", 0, 3), ("out", 0, 2, True)],
                [("qkv", 1, 2, False), ("attn", 1, 0), ("out", 0, 3, False)],
                [("qkv", 1, 3, False), ("attn", 1, 1), ("out", 1, 0, False)],
                [("attn", 1, 3), ("out", 1, 1, "vec")],
                [("attn", 1, 2), ("out", 1, 3, "vec")],
                [("out", 1, 2, "vec")],
            ]
            for bi, bracket in enumerate(brackets):
                streams = []
                for item in bracket:
                    if item[0] == "qkv":
                        _, b, scb, piecewise = item
                        sc = b * NQC + scb
                        if sc + 2 < NSC_G:
                            xt_sls[sc + 2], _ = emit_xt_dma(sc + 2,
                                                            piecewise=False)
                        streams.append(
                            qkv_steps(b, scb, xt_sls[sc], piecewise))
                    elif item[0] == "attn":
                        streams.append(attn_steps(item[1], item[2]))
                    else:
                        streams.append(outproj_steps(
                            item[1], item[2], oe_scalar=(item[3] is True),
                            oe_vector=(item[3] == "vec"),
                            tail=(bi == len(brackets) - 1)))
                _weave(streams)

    nc.finalize()
    return nc


_NC = None


def _get_nc() -> bass.Bass:
    global _NC
    if _NC is None:
        _NC = _build_nc()
    return _NC


def _make_in_maps(resid_pre, Wq, Wk, Wv, Wo):
    bf = ml_dtypes.bfloat16
    x = np.asarray(resid_pre, np.float32).reshape(BS, D)
    # pre-tiled DMA-friendly layout: xt[sc, p, dt*512 + s] = x[sc*512+s, dt*128+p]
    xt = np.ascontiguousarray(
        x.reshape(NSC_G, 512, D // 128, 128).transpose(0, 3, 2, 1).reshape(NSC_G, 128, 8192)
    ).astype(bf)

    p = np.arange(128)[:, None]
    f = np.arange(512)[None, :]

    Wq = np.asarray(Wq, np.float32)
    Wk = np.asarray(Wk, np.float32)
    Wv = np.asarray(Wv, np.float32)
    Wo = np.asarray(Wo, np.float32)

    in_maps = []
    for c in range(NCORES):
        rows = np.r_[c * DH:(c + 1) * DH, (c + 8) * DH:(c + 9) * DH]
        s0 = _SLOPES[c]       # slot0 slope (exact per-q shift convention)
        s1 = _SLOPES[c + 8]   # slot1 slope (per-chunk constant shift)
        qrow = np.zeros((128, 4 * 512), np.float32)
        kbias = np.zeros((128, 17 + 64), np.float32)
        mask = np.zeros((128, 2 * 512), np.float32)
        for qc in range(4):
            q = qc * 512 + np.arange(512, dtype=np.float64)
            qrow[:, qc * 512:(qc + 1) * 512] = (-s0 * q / SCALE)[None, :].astype(np.float32)
        for kt in range(16):
            kbias[:, kt] = (
                s0 * (kt * 128 + np.arange(128, dtype=np.float64)) - C0
            ).astype(np.float32)
        kbias[:, 16] = (s0 * np.arange(128, dtype=np.float64) - C0).astype(np.float32)
        # slot0 diagonal-band mask: exp arg = scale*s + s0*(p - q'') - C0
        mask[:, 0:512] = ((-s0 * f / SCALE) + np.where(p > f, NEG, 0.0)).astype(np.float32)
        # slot1: causal handled post-exp by affine_select; alibi fully in
        # per-(qc,kt) bias with chunk-end shift
        mask[:, 512:1024] = np.where(p > f, NEG, 0.0).astype(np.float32)
        for qc in range(4):
            for kt in range(4 * qc + 4):
                kbias[:, 17 + qc * 16 + kt] = (
                    s1 * (kt * 128 + np.arange(128, dtype=np.float64)
                          - (qc * 512 + 511)) - C0
                ).astype(np.float32)
        in_maps.append({
            "xt": xt,
            # [p, dt*HD + m] = W.T[dt*128+p, m]  (contiguous 8KB rows)
            "wq_t": np.ascontiguousarray(
                Wq[rows, :].T.reshape(D // 128, 128, HD).transpose(1, 0, 2).reshape(128, -1)
            ).astype(bf),
            "wk_t": np.ascontiguousarray(
                Wk[rows, :].T.reshape(D // 128, 128, HD).transpose(1, 0, 2).reshape(128, -1)
            ).astype(bf),
            "wv_t": np.ascontiguousarray(
                Wv[rows, :].T.reshape(D // 128, 128, HD).transpose(1, 0, 2).reshape(128, -1)
            ).astype(bf),
            # [p, lh*D + o] = Wo[:, rows].T[lh*128+p, o]
            "wo_t": np.ascontiguousarray(
                Wo[:, rows].T.reshape(HL, 128, D).transpose(1, 0, 2).reshape(128, -1)
            ).astype(bf),
            "mask": mask,
            "qrow": qrow,
            "kbias": kbias,
        })
    return in_maps


def _combine(results) -> np.ndarray:
    acc = np.zeros((NSC_G, 128, 16, 512), np.float32)
    for m in results:
        acc += m["out_t"].astype(np.float32)
    # out[ot*128+p, sc*512+c] = acc[sc, p, ot, c]
    full = np.ascontiguousarray(acc.transpose(2, 1, 0, 3).reshape(D, BS))
    return np.ascontiguousarray(full.reshape(D, B, S).transpose(1, 2, 0))


def kernel(resid_pre, Wq, Wk, Wv, Wo):
    nc = _get_nc()
    in_maps = _make_in_maps(resid_pre, Wq, Wk, Wv, Wo)
    res = run_bass_kernel_spmd(nc, in_maps, core_ids=list(range(NCORES)))
    return _combine(res.results)
